# revision 1
# baseline (speedup 1.0000x reference)
"""Trainium2 Bass kernel for fused QKV projection + interleaved RoPE.

Problem: X[4, 4096, 2048] @ {Wq, Wk, Wv}[2048, 2048] -> reshape to heads
[B, S, 16, 128], apply interleaved RoPE to Q and K, return (Xq, Xk, Xv).

Sharding: data-parallel over tokens. The 4*4096 = 16384 token rows are
split into 8 contiguous shards of 2048 rows (core c gets batch c//2,
sequence half c%2). Every core holds the full Wq/Wk/Wv and computes all
2048 output features for its rows; RoPE is per-token elementwise so no
communication is needed.

Device kernel (identical SPMD program on all 8 cores):
  - X^T shard (cast to bf16 on host) stays resident in SBUF as 16
    per-row-chunk tiles; weights stream through double-buffered half-M
    tiles so each of the six (tensor, m-half) phases prefetches the next.
  - matmul out = lhsT.T @ rhs with lhsT = X^T tile [128k, 128r]
    (stationary) and rhs = W tile [128k, 512m] (moving), accumulating
    psum[128r, 1024m] fp32 over 16 k-chunks.
  - RoPE in 3 DVE ops on the psum tile: the interleaved pair swap is a
    reversed-stride access pattern, the rotation sign is pre-baked into
    the sin table on the host, and cos/sin broadcast across heads via
    zero-stride APs. V is copied back on the scalar engine.
"""

import numpy as np
import ml_dtypes

import concourse.bass as bass
import concourse.mybir as mybir
import concourse.tile as tile
from concourse import bacc
from concourse.bass import ds, ts
from concourse.bass_utils import run_bass_kernel_spmd

B, S, DIM, H = 4, 4096, 2048, 16
HD = DIM // H           # 128
N_CORES = 8
R = B * S // N_CORES    # 2048 token rows per core
P = 128

BF16 = mybir.dt.bfloat16
F32 = mybir.dt.float32


def build_nc(K=DIM, M=DIM, rows=R, hd=HD, mm_free=512, m_half=1024, loop_n=1,
             unroll=False):
    """Build the per-core Bass program.

    K: contraction dim, M: output feature dim, rows: token rows per core.
    loop_n > 1 wraps the body in a device-side For_i for benchmarking.
    """
    m_half = min(m_half, M)
    assert K % P == 0 and rows % P == 0 and M % m_half == 0
    assert m_half % mm_free == 0 and m_half % hd == 0
    KO = K // P           # k-chunks
    RC = rows // P        # token row chunks
    HALVES = M // m_half  # weight column phases per tensor
    MJ = m_half // mm_free
    NH = m_half // hd     # heads per column phase
    J = hd // 2           # rotation pairs per head

    nc = bacc.Bacc(None, target_bir_lowering=False)

    xt = nc.dram_tensor("xt", [K, rows], BF16, kind="ExternalInput")
    wq = nc.dram_tensor("wq", [K, M], BF16, kind="ExternalInput")
    wk = nc.dram_tensor("wk", [K, M], BF16, kind="ExternalInput")
    wv = nc.dram_tensor("wv", [K, M], BF16, kind="ExternalInput")
    cosf = nc.dram_tensor("cosf", [rows, hd], F32, kind="ExternalInput")
    ssin = nc.dram_tensor("ssin", [rows, hd], F32, kind="ExternalInput")
    q_out = nc.dram_tensor("q", [rows, M], F32, kind="ExternalOutput")
    k_out = nc.dram_tensor("k", [rows, M], F32, kind="ExternalOutput")
    v_out = nc.dram_tensor("v", [rows, M], F32, kind="ExternalOutput")

    xt_r = xt[:].rearrange("(ko p) r -> p ko r", p=P)
    cos_r = cosf[:].rearrange("(rc p) d -> p rc d", p=P)
    sin_r = ssin[:].rearrange("(rc p) d -> p rc d", p=P)

    with tile.TileContext(nc) as tc:
        with (
            tc.tile_pool(name="wpool", bufs=2 * (K // P)) as wpool,
            tc.tile_pool(name="xpool", bufs=RC) as xpool,
            tc.tile_pool(name="cpool", bufs=1) as cpool,
            tc.tile_pool(name="opool", bufs=4) as opool,
            tc.tile_pool(name="tpool", bufs=2) as tpool,
            tc.tile_pool(name="psum", bufs=4, space="PSUM") as pspool,
        ):
            def load_w_tiles(w_r, half):
                # per-ko tiles so the first matmul only waits on 256 KB
                tiles = []
                for ko in range(KO):
                    w_sb = wpool.tile([P, m_half], BF16, tag="w")
                    nc.scalar.dma_start(w_sb[:], w_r[:, ko, ts(half, m_half)])
                    tiles.append(w_sb)
                return tiles

            def lhsT_of(xt_tiles, rc, ko):
                xt = xt_tiles[rc]
                if isinstance(xt, list):  # ko-chunked tile list
                    per = KO // len(xt)
                    return xt[ko // per][:, ko % per]
                return xt[:, ko]

            def emit_phase(w_tiles, o_r, half, rope, xt_tiles, cos_sb, sin_sb,
                           pair0=False):
                start_rc = 0
                if pair0 and RC >= 2:
                    # First k-sweep alone consumes W tiles ~1.6x faster than
                    # HBM delivers them; interleave rc0+rc1 (2 psums live,
                    # same tiles) so each W tile feeds 4 matmuls.
                    ps0 = pspool.tile([P, m_half], F32, tag="ps", name="ps_p0")
                    ps1 = pspool.tile([P, m_half], F32, tag="ps", name="ps_p1")
                    for ko in range(KO):
                        for psx, rc in ((ps0, 0), (ps1, 1)):
                            for mj in range(MJ):
                                nc.tensor.matmul(
                                    psx[:, ts(mj, mm_free)],
                                    lhsT_of(xt_tiles, rc, ko),
                                    w_tiles[ko][:, ts(mj, mm_free)],
                                    start=(ko == 0),
                                    stop=(ko == KO - 1),
                                )
                    finish_rc(ps0, o_r, half, 0, rope, cos_sb, sin_sb)
                    finish_rc(ps1, o_r, half, 1, rope, cos_sb, sin_sb)
                    start_rc = 2
                for rc in range(start_rc, RC):
                    psum = pspool.tile([P, m_half], F32, tag="ps")
                    for ko in range(KO):
                        for mj in range(MJ):
                            nc.tensor.matmul(
                                psum[:, ts(mj, mm_free)],
                                lhsT_of(xt_tiles, rc, ko),
                                w_tiles[ko][:, ts(mj, mm_free)],
                                start=(ko == 0),
                                stop=(ko == KO - 1),
                            )
                    finish_rc(psum, o_r, half, rc, rope, cos_sb, sin_sb)

            def finish_rc(psum, o_r, half, rc, rope, cos_sb, sin_sb,
                          c0=0, mc=None):
                    mc = m_half if mc is None else mc
                    nh = mc // hd
                    ps = psum[:, ds(c0, mc)]
                    o_sb = opool.tile([P, mc], F32, tag="o")
                    if rope:
                        # o = x*cos + swap_pairs(x)*ssin; ssin sign-baked,
                        # the swap is a reversed-stride AP on the pair dim.
                        ps_hd = ps.rearrange("p (h d) -> p h d", d=hd)
                        ps_pr = ps.rearrange(
                            "p (h j two) -> p h j two", h=nh, two=2
                        )
                        cos_b = cos_sb[:, rc, None, :].to_broadcast([P, nh, hd])
                        sin_b = sin_sb[:, rc].rearrange(
                            "p (j two) -> p j two", two=2
                        )[:, None, :, :].to_broadcast([P, nh, J, 2])

                        t_sb = tpool.tile([P, mc], F32, tag="t")
                        t_pr = t_sb[:].rearrange(
                            "p (h j two) -> p h j two", h=nh, two=2
                        )
                        o_hd = o_sb[:].rearrange("p (h d) -> p h d", d=hd)

                        nc.vector.tensor_tensor(
                            t_pr[:], ps_pr[:, :, :, ::-1], sin_b,
                            mybir.AluOpType.mult,
                        )
                        nc.vector.tensor_tensor(
                            o_hd, ps_hd, cos_b, mybir.AluOpType.mult,
                        )
                        nc.vector.tensor_tensor(
                            o_sb[:], o_sb[:], t_sb[:], mybir.AluOpType.add,
                        )
                    else:
                        nc.scalar.copy(o_sb[:], ps)

                    # stores share the ACT HWDGE ring with the (small,
                    # interleaved) weight prefetches; activations + freqs
                    # own the SP ring so neither queue head-of-line blocks
                    nc.scalar.dma_start(
                        o_r[:, rc, ds(half * m_half + c0, mc)], o_sb[:])

            def body():
                # Cold-start ordering: the first matmuls need only x[0] and
                # the first W tiles, so issue those before everything else
                # (x on the SP HWDGE ring, W on ACT's).
                phases = []
                for w_dram, o_dram, rope in (
                    (wv, v_out, False),  # V first: no RoPE, so the cold
                    (wq, q_out, True),   # start has no cos/sin dependency
                    (wk, k_out, True),
                ):
                    w_r = w_dram[:].rearrange("(ko p) m -> p ko m", p=P)
                    o_r = o_dram[:].rearrange("(rc p) m -> p rc m", p=P)
                    for half in range(HALVES):
                        phases.append((w_r, o_r, half, rope))

                x0 = xpool.tile([P, KO, P], BF16, tag="x")
                nc.sync.dma_start(x0[:], xt_r[:, :, ts(0, P)])
                w_first = load_w_tiles(phases[0][0], phases[0][2])

                xt_tiles = [x0]
                cos_sb = sin_sb = None
                for rc in range(1, RC):
                    x_sb = xpool.tile([P, KO, P], BF16, tag="x")
                    nc.sync.dma_start(x_sb[:], xt_r[:, :, ts(rc, P)])
                    xt_tiles.append(x_sb)
                    if rc == min(3, RC - 1):
                        cos_sb = cpool.tile([P, RC, hd], F32, tag="cos")
                        sin_sb = cpool.tile([P, RC, hd], F32, tag="sin")
                        nc.sync.dma_start(cos_sb[:], cos_r)
                        nc.sync.dma_start(sin_sb[:], sin_r)
                if cos_sb is None:
                    cos_sb = cpool.tile([P, RC, hd], F32, tag="cos")
                    sin_sb = cpool.tile([P, RC, hd], F32, tag="sin")
                    nc.sync.dma_start(cos_sb[:], cos_r)
                    nc.sync.dma_start(sin_sb[:], sin_r)

                for i, (w_r, o_r, half, rope) in enumerate(phases):
                    w_tiles = w_first if i == 0 else load_w_tiles(w_r, half)
                    emit_phase(w_tiles, o_r, half, rope, xt_tiles, cos_sb,
                               sin_sb, pair0=(i == 0))

            if loop_n == 1:
                body()
            elif unroll:
                for _ in range(loop_n):
                    body()
            else:
                with tc.For_i(0, loop_n, 1):
                    body()

    nc.compile()
    return nc


_NC_CACHE = {}


def _get_nc():
    if "nc" not in _NC_CACHE:
        _NC_CACHE["nc"] = build_nc()
    return _NC_CACHE["nc"]


def prepare_in_maps(X, freqs_cos, freqs_sin, Wq, Wk, Wv):
    X = np.asarray(X, dtype=np.float32)
    freqs_cos = np.asarray(freqs_cos, dtype=np.float32)
    freqs_sin = np.asarray(freqs_sin, dtype=np.float32)

    Xf = X.reshape(B * S, DIM)
    Xb = Xf.astype(ml_dtypes.bfloat16)
    wq_b = np.asarray(Wq, dtype=np.float32).astype(ml_dtypes.bfloat16)
    wk_b = np.asarray(Wk, dtype=np.float32).astype(ml_dtypes.bfloat16)
    wv_b = np.asarray(Wv, dtype=np.float32).astype(ml_dtypes.bfloat16)

    # Rotation sign baked into sin: out[2i] = x[2i]c - x[2i+1]s,
    # out[2i+1] = x[2i+1]c + x[2i]s.
    ssin_full = freqs_sin.copy()
    ssin_full[:, 0::2] *= -1.0

    in_maps = []
    for c in range(N_CORES):
        rows = slice(c * R, (c + 1) * R)
        s0 = (c % 2) * R  # sequence offset of this shard (R == S // 2)
        in_maps.append({
            "xt": np.ascontiguousarray(Xb[rows].T),
            "wq": wq_b,
            "wk": wk_b,
            "wv": wv_b,
            "cosf": np.ascontiguousarray(freqs_cos[s0:s0 + R]),
            "ssin": np.ascontiguousarray(ssin_full[s0:s0 + R]),
        })
    return in_maps


def assemble_outputs(results):
    Xq = np.empty((B * S, H, HD), dtype=np.float32)
    Xk = np.empty((B * S, H, HD), dtype=np.float32)
    Xv = np.empty((B * S, H, HD), dtype=np.float32)
    for c in range(N_CORES):
        rows = slice(c * R, (c + 1) * R)
        Xq[rows] = results[c]["q"].reshape(R, H, HD)
        Xk[rows] = results[c]["k"].reshape(R, H, HD)
        Xv[rows] = results[c]["v"].reshape(R, H, HD)

    return (
        Xq.reshape(B, S, H, HD),
        Xk.reshape(B, S, H, HD),
        Xv.reshape(B, S, H, HD),
    )


def kernel(X, freqs_cos, freqs_sin, attention_mask, Wq, Wk, Wv):
    in_maps = prepare_in_maps(X, freqs_cos, freqs_sin, Wq, Wk, Wv)
    nc = _get_nc()
    res = run_bass_kernel_spmd(nc, in_maps, list(range(N_CORES)))
    return assemble_outputs(res.results)

